# revision 4
# baseline (speedup 1.0000x reference)
"""Trainium2 Bass kernel for nn_MlpwithSOMModule (sum-of-max hard-attention score).

Math identity used: the reference computes
    sim  = ctx @ ent^T            # [L, M] per (b, k)
    idx  = argmax_m sim
    out  = sum_l dot(ctx_l, ent_idx[l]) = sum_l sim[l, idx[l]] = sum_l max_m sim[l, m]
so no gather/argmax is needed on device - just matmul, row-max, and a sum.

Sharding: B = 8 == n_cores, so core c processes context[c] = [64, 2, 256, 768]
(its 64 (b,k) pairs are exactly batch b == c). No cross-core communication.

Per-core pipeline (Tile framework):
  1. SWDGE DMA loads 4 pairs at a time, casting fp32 -> bf16 in the DMA
     (natural [token, d] layout, fully contiguous reads).
  2. PE transposes 24 [128,128] bf16 blocks per pair (ctx and ent to [d, token])
     into bf16 PSUM banks; ACT/DVE copy 1024-wide slabs back to SBUF.
  3. 12 accumulating bf16 matmuls per pair: S[l, m] (contraction d = 6 x 128).
  4. DVE reduce_max over m -> row maxes collected in a [128, 128] SBUF tile.
  5. One final fp32 matmul with a ones-vector sums over l (partition dim),
     then a strided DVE add folds the two l-chunks; DMA out [1, 64].
"""

import sys

for _p in ("/opt/trn_rl_repo", "/root/.axon_site/_ro/trn_rl_repo"):
    if _p not in sys.path:
        sys.path.insert(0, _p)

import numpy as np

B, TOPK, L, D = 8, 64, 256, 768
N_CORES = 8
PAIRS_PER_CORE = 64  # == TOPK; one batch index per core
GROUP = 4  # pairs loaded per DMA
P = 128
DCHUNKS = D // P  # 6
LCHUNKS = L // P  # 2

_cache = {}


def _build():
    import concourse.bass as bass
    import concourse.mybir as mybir
    from concourse import bacc
    from concourse.tile import TileContext
    from concourse.masks import make_identity

    nc = bacc.Bacc(
        "TRN2",
        target_bir_lowering=False,
        debug=False,
        num_devices=N_CORES,
    )

    x = nc.dram_tensor(
        "x", [PAIRS_PER_CORE, 2, L, D], mybir.dt.float32, kind="ExternalInput"
    ).ap()
    out = nc.dram_tensor(
        "out", [1, PAIRS_PER_CORE], mybir.dt.float32, kind="ExternalOutput"
    ).ap()

    bf16 = mybir.dt.bfloat16
    f32 = mybir.dt.float32
    NGROUPS = PAIRS_PER_CORE // GROUP
    CPG = GROUP * 4  # [128, d] chunks per group (pairs * 2 slices * 2 l-chunks)

    # DRAM view: group g, partition p(=token%128), chunk c=(q,s,lc), d
    xr = x.rearrange("(g q) s (lc p) d -> g p (q s lc) d", q=GROUP, p=P)

    with TileContext(nc) as tc:
        with (
            tc.tile_pool(name="const", bufs=1) as cpool,
            tc.tile_pool(name="xload", bufs=3) as xpool,
            tc.tile_pool(name="tpose", bufs=3) as tpool,
            tc.tile_pool(name="ppose", bufs=5, space="PSUM") as ppool,
            tc.tile_pool(name="pmm", bufs=2, space="PSUM") as mpool,
            tc.tile_pool(name="pfin", bufs=1, space="PSUM") as fpool,
        ):
            ident = cpool.tile([P, P], bf16)
            make_identity(nc, ident)
            ones = cpool.tile([P, 1], f32)
            nc.gpsimd.memset(ones, 1.0)
            # row maxes: column 2*pair+lc holds max_m S[l, m] for l-chunk lc
            RM = cpool.tile([P, 2 * PAIRS_PER_CORE], f32)

            def emit_mm(pair, T):
                ps = mpool.tile([P, LCHUNKS, 2 * P], f32)
                for lc in range(LCHUNKS):
                    for dc in range(DCHUNKS):
                        off = (dc * 2 + lc) * P
                        nc.tensor.matmul(
                            ps[:, lc],
                            T[:, off : off + P],  # ctxT block [d, l-chunk]
                            T[:, 1536 + dc * 2 * P : 1536 + (dc + 1) * 2 * P],
                            start=(dc == 0),
                            stop=(dc == DCHUNKS - 1),
                        )
                nc.vector.reduce_max(
                    RM[:, 2 * pair : 2 * pair + 2], ps, axis=mybir.AxisListType.X
                )

            prev = None
            for g in range(NGROUPS):
                X = xpool.tile([P, CPG, D], bf16)
                nc.gpsimd.dma_start(X, xr[g])  # fp32 -> bf16 cast in DMA
                for q in range(GROUP):
                    pair = g * GROUP + q
                    # T: ctxT at [0, 1536), entT at [1536, 3072); block (t, dc, lc)
                    # lives at free offset 128*(t*12 + dc*2 + lc)
                    T = tpool.tile([P, 2 * 1536], bf16)
                    for jj in range(3):
                        psb = ppool.tile([P, 1024], bf16)
                        for slot in range(8):
                            j = jj * 8 + slot
                            t, rem = divmod(j, 12)
                            dc, lc = divmod(rem, 2)
                            c = q * 4 + t * 2 + lc
                            nc.tensor.transpose(
                                psb[:, slot * P : (slot + 1) * P],
                                X[:, c, dc * P : (dc + 1) * P],
                                ident,
                            )
                        nc.any.tensor_copy(T[:, jj * 1024 : (jj + 1) * 1024], psb)
                    if prev is not None:
                        emit_mm(*prev)
                    prev = (pair, T)
            emit_mm(*prev)

            # out[pair] = sum over l = sum over 128 partitions of both lc columns
            fin = fpool.tile([1, 2 * PAIRS_PER_CORE], f32)
            nc.tensor.matmul(fin, ones, RM, start=True, stop=True)
            fsb = cpool.tile([1, 2 * PAIRS_PER_CORE], f32)
            nc.vector.tensor_copy(fsb, fin)
            osb = cpool.tile([1, PAIRS_PER_CORE], f32)
            fsb2 = fsb.rearrange("p (n two) -> p n two", two=2)
            nc.vector.tensor_tensor(
                osb, fsb2[:, :, 0], fsb2[:, :, 1], op=mybir.AluOpType.add
            )
            nc.sync.dma_start(out, osb)

    nc.compile()
    return nc


def _get_nc():
    if "nc" not in _cache:
        _cache["nc"] = _build()
    return _cache["nc"]


def run(context, trace=False, tmpdir=None):
    from concourse import bass_utils

    nc = _get_nc()
    context = np.ascontiguousarray(np.asarray(context, dtype=np.float32))
    assert context.shape == (B, TOPK, 2, L, D), context.shape
    in_maps = [{"x": context[c]} for c in range(N_CORES)]
    res = bass_utils.run_bass_kernel_spmd(
        nc, in_maps, core_ids=list(range(N_CORES)), trace=trace, tmpdir=tmpdir
    )
    out = np.concatenate(
        [res.results[c]["out"].reshape(1, PAIRS_PER_CORE) for c in range(N_CORES)],
        axis=0,
    ).astype(np.float32)
    return out, res


def kernel(context):
    out, _ = run(context, trace=False)
    return out


# revision 8
# speedup vs baseline: 1.0847x; 1.0847x over previous
"""Trainium2 Bass kernel for nn_MlpwithSOMModule (sum-of-max hard-attention score).

Math identity used: the reference computes
    sim  = ctx @ ent^T            # [L, M] per (b, k)
    idx  = argmax_m sim
    out  = sum_l dot(ctx_l, ent_idx[l]) = sum_l sim[l, idx[l]] = sum_l max_m sim[l, m]
so no gather/argmax is needed on device - just matmul, row-max, and a sum.

Sharding: B = 8 == n_cores, so core c processes context[c] = [64, 2, 256, 768]
(its 64 (b,k) pairs are exactly batch b == c). No cross-core communication.

Per-core pipeline (Tile framework):
  1. SWDGE DMA loads 4 pairs at a time, casting fp32 -> bf16 in the DMA
     (natural [token, d] layout, fully contiguous reads).
  2. PE transposes 24 [128,128] bf16 blocks per pair (ctx and ent to [d, token])
     into bf16 PSUM banks; ACT/DVE copy 1024-wide slabs back to SBUF.
  3. 12 accumulating bf16 matmuls per pair: S[l, m] (contraction d = 6 x 128).
  4. DVE reduce_max over m -> row maxes collected in a [128, 128] SBUF tile.
  5. One final fp32 matmul with a ones-vector sums over l (partition dim),
     then a strided DVE add folds the two l-chunks; DMA out [1, 64].
"""

import sys

for _p in ("/opt/trn_rl_repo", "/root/.axon_site/_ro/trn_rl_repo"):
    if _p not in sys.path:
        sys.path.insert(0, _p)

import numpy as np

B, TOPK, L, D = 8, 64, 256, 768
N_CORES = 8
PAIRS_PER_CORE = 64  # == TOPK; one batch index per core
GROUP = 4  # pairs loaded per DMA
P = 128
DCHUNKS = D // P  # 6
LCHUNKS = L // P  # 2

_cache = {}


def _build():
    import concourse.bass as bass
    import concourse.mybir as mybir
    from concourse import bacc
    from concourse.tile import TileContext
    from concourse.masks import make_identity

    nc = bacc.Bacc(
        "TRN2",
        target_bir_lowering=False,
        debug=False,
        num_devices=N_CORES,
    )

    x = nc.dram_tensor(
        "x", [PAIRS_PER_CORE, 2, L, D], mybir.dt.float32, kind="ExternalInput"
    ).ap()
    out = nc.dram_tensor(
        "out", [1, PAIRS_PER_CORE], mybir.dt.float32, kind="ExternalOutput"
    ).ap()

    bf16 = mybir.dt.bfloat16
    f32 = mybir.dt.float32

    # Load plan: small loads at the head (compute starts sooner) and tail
    # (last pair's compute isn't stuck behind a 6MB load), 4-pair loads in
    # the middle for DMA efficiency.
    sizes = [1, 1, 2] + [GROUP] * 14 + [2, 1, 1]
    assert sum(sizes) == PAIRS_PER_CORE
    loads = []
    s0 = 0
    for n in sizes:
        loads.append((s0, n))
        s0 += n

    # DRAM view: pair pr, partition p(=token%128), chunk c=(s,lc), d
    xv = x.rearrange("pr s (lc p) d -> pr p (s lc) d", p=P)

    with TileContext(nc) as tc:
        with (
            tc.tile_pool(name="const", bufs=1) as cpool,
            tc.tile_pool(name="xload", bufs=3) as xpool,
            tc.tile_pool(name="tpose", bufs=3) as tpool,
            tc.tile_pool(name="ppose", bufs=5, space="PSUM") as ppool,
            tc.tile_pool(name="pmm", bufs=2, space="PSUM") as mpool,
            tc.tile_pool(name="pfin", bufs=1, space="PSUM") as fpool,
        ):
            ident = cpool.tile([P, P], bf16)
            ones = cpool.tile([P, 1], f32)
            # row maxes: column 2*pair+lc holds max_m S[l, m] for l-chunk lc
            RM = cpool.tile([P, 2 * PAIRS_PER_CORE], f32)

            def emit_mm(pair, T):
                ps = mpool.tile([P, LCHUNKS, 2 * P], f32)
                for lc in range(LCHUNKS):
                    for dc in range(DCHUNKS):
                        off = (dc * 2 + lc) * P
                        nc.tensor.matmul(
                            ps[:, lc],
                            T[:, off : off + P],  # ctxT block [d, l-chunk]
                            T[:, 1536 + dc * 2 * P : 1536 + (dc + 1) * 2 * P],
                            start=(dc == 0),
                            stop=(dc == DCHUNKS - 1),
                        )
                nc.vector.reduce_max(
                    RM[:, 2 * pair : 2 * pair + 2], ps, axis=mybir.AxisListType.X
                )

            prev = None
            for li, (p0, n) in enumerate(loads):
                Xfull = xpool.tile([P, 4 * GROUP, D], bf16, tag="X", name="X")
                X = Xfull[:, : 4 * n, :]
                # fp32 -> bf16 cast in DMA (SWDGE)
                nc.gpsimd.dma_start(X, xv[p0 : p0 + n].rearrange("n p c d -> p (n c) d"))
                if li == 0:
                    # emitted after the first dma_start so the Q7/SWDGE engine
                    # starts descriptor generation immediately at kernel start
                    make_identity(nc, ident)
                    nc.gpsimd.memset(ones, 1.0)
                for q in range(n):
                    pair = p0 + q
                    if prev is not None:
                        emit_mm(*prev)
                    # T: ctxT at [0, 1536), entT at [1536, 3072); block (t, dc, lc)
                    # lives at free offset 128*(t*12 + dc*2 + lc)
                    T = tpool.tile([P, 2 * 1536], bf16)
                    for jj in range(3):
                        psb = ppool.tile([P, 1024], bf16)
                        for slot in range(8):
                            j = jj * 8 + slot
                            t, rem = divmod(j, 12)
                            dc, lc = divmod(rem, 2)
                            c = q * 4 + t * 2 + lc
                            nc.tensor.transpose(
                                psb[:, slot * P : (slot + 1) * P],
                                X[:, c, dc * P : (dc + 1) * P],
                                ident,
                            )
                        nc.any.tensor_copy(T[:, jj * 1024 : (jj + 1) * 1024], psb)
                    prev = (pair, T)
            emit_mm(*prev)

            # out[pair] = sum over l = sum over 128 partitions of both lc columns
            fin = fpool.tile([1, 2 * PAIRS_PER_CORE], f32)
            nc.tensor.matmul(fin, ones, RM, start=True, stop=True)
            fsb = cpool.tile([1, 2 * PAIRS_PER_CORE], f32)
            nc.vector.tensor_copy(fsb, fin)
            osb = cpool.tile([1, PAIRS_PER_CORE], f32)
            fsb2 = fsb.rearrange("p (n two) -> p n two", two=2)
            nc.vector.tensor_tensor(
                osb, fsb2[:, :, 0], fsb2[:, :, 1], op=mybir.AluOpType.add
            )
            nc.sync.dma_start(out, osb)

    nc.compile()
    return nc


def _get_nc():
    if "nc" not in _cache:
        _cache["nc"] = _build()
    return _cache["nc"]


def run(context, trace=False, tmpdir=None):
    from concourse import bass_utils

    nc = _get_nc()
    context = np.ascontiguousarray(np.asarray(context, dtype=np.float32))
    assert context.shape == (B, TOPK, 2, L, D), context.shape
    in_maps = [{"x": context[c]} for c in range(N_CORES)]
    res = bass_utils.run_bass_kernel_spmd(
        nc, in_maps, core_ids=list(range(N_CORES)), trace=trace, tmpdir=tmpdir
    )
    out = np.concatenate(
        [res.results[c]["out"].reshape(1, PAIRS_PER_CORE) for c in range(N_CORES)],
        axis=0,
    ).astype(np.float32)
    return out, res


def kernel(context):
    out, _ = run(context, trace=False)
    return out
